# revision 35
# baseline (speedup 1.0000x reference)
"""Multi-head causal attention (d_model=768, 12 heads, seq 2048, batch 2) on
8 Trainium2 NeuronCores.

Sharding: tensor-parallel over heads x data-parallel over batch.
Core c handles batch b = c // 4 and heads [3*(c%4), 3*(c%4)+3).
Each core computes its 3 heads' attention plus its partial output
projection; the host sums the 4 partials per batch and adds the bias.

Design (all-bf16 PE datapath), ~115us/core measured:
  - Scores matmuls have only K=64 contraction, so two are packed into
    the 128-row array concurrently via tile_position row groups. Heads
    0,1 pair naturally; head 2 pairs its own chunks j/j+1 against
    partition-duplicated Q2/K2 copies.  Score pairs are emitted
    clustered two-at-a-time, two pairs ahead of the exp stream: each
    cluster pays the row-group LDWEIGHTS serialization (~100ns, no
    background-buffer pull-ahead for tile_position loads) once.
  - Software-pipelined emission: P.V runs one pair behind exp; QKV for
    the next 512-block and output-projection halves are interleaved
    between pairs as PE fillers with per-sup quotas chosen so each
    superblock's PE load matches its ACT load.  The PE never idles
    long enough for the HAM activity monitor to re-throttle the clock
    to 1.2 GHz (it stays at 2.4 GHz for the whole kernel body).
  - All attention-side operands (q01/k01/q2d/k2d/V/ot01/ot2) are split
    into per-512-block tiles so filler writes never create coarse
    whole-tile false dependencies against pair-stream reads.
  - exp runs as one ACTIVATE per pair ([128,2,512] PSUM); diagonal
    triangles are zeroed after exp by gpsimd affine_select.  All
    partition_broadcasts are emitted after the affine_selects they
    would otherwise clog, and a dummy broadcast at build start forces
    the gpsimd "attn" ucode library load (~7us) into the DMA wait.
    gpsimd.tensor_copy is avoided entirely (it lives in the "standard"
    library; mixing would thrash a ~6us library reload per switch).
  - Row sums l come from a ones-column appended to V ([V|1]).
    Normalization per head: copy the l row, reciprocal_approx_fast,
    partition_broadcast (deferred), then one tensor_mul reading O
    directly from PSUM (no staging copy; the read frees the bank).
  - PSUM budget: score pairs 2x[128,2,512] (4 banks) + 3 P.V
    accumulators + 1 filler bank = 8.  V accumulation is block-outer:
    interleaved accumulation groups sharing a PSUM bank corrupt each
    other.
  - Inputs are host-prepped per-512-block chunk-major and DMAed
    critical-first on both HWDGE queues (xT t-block-0 per-chunk so the
    first QKV matmul starts ~3us after the rings open); warm-up
    matmuls hold the PE busy through the DMA wait so qkv0 runs at
    full clock.
  - Tail: the last superblock's h2 norm chain is bridged with
    keepalive matmuls plus ot01-half preloads of the final OP blocks
    into the freed score banks; the scalar engine (exp done) takes
    half the final evacuations and output DMAs.
"""

import sys
import types

import numpy as np
import ml_dtypes

import concourse.bass as bass
import concourse.tile as tile
from concourse import mybir, bacc
from concourse.bass_utils import run_bass_kernel_spmd

# Register the axon NTFF profiling hook if the environment supports it, so
# running with BASS_TRACE=1 yields exec_time_ns instead of an import error.
try:
    import antenv.axon_hooks  # noqa: F401
except ImportError:
    try:
        from trn_agent_boot.trn_boot import _ntff_profile_via_ctypes

        _hook = _ntff_profile_via_ctypes("/opt/axon/libaxon_pjrt.so")
        _mod = types.ModuleType("antenv.axon_hooks")
        _mod.get_axon_ntff_profile_hook = lambda: _hook
        _mod.set_axon_ntff_profile_hook = lambda h: None
        sys.modules["antenv.axon_hooks"] = _mod
    except Exception:
        pass

F32 = mybir.dt.float32
BF16 = mybir.dt.bfloat16

N_CORES = 8
B = 2
S = 2048
D = 768
H = 12
DK = 64
H_PER_CORE = 3
NSUP = S // 512  # 4 q super-blocks of 512
NKCH = D // 128  # 6 contraction chunks
SCALE = 0.125  # 1/sqrt(64)

_CACHED_NC = None


def build_bass():
    nc = bacc.Bacc()
    # inputs, chunk-major per 512-col t-block: xtb*[k][128, 512]
    xtb0 = [nc.declare_dram_parameter(f"xtb0_{k}", [128, 512], BF16,
                                      isOutput=False) for k in range(6)]
    xtb1 = nc.declare_dram_parameter("xtb1", [128, 6, 512], BF16, isOutput=False)
    xtb2 = nc.declare_dram_parameter("xtb2", [128, 6, 512], BF16, isOutput=False)
    xtb3 = nc.declare_dram_parameter("xtb3", [128, 6, 512], BF16, isOutput=False)
    # wqk chunk columns: [Wq0|Wq1], [Wk0|Wk1], [Wq2|Wk2]
    wqka = nc.declare_dram_parameter("wqka", [128, 3, 384], BF16, isOutput=False)
    wqkb = nc.declare_dram_parameter("wqkb", [128, 3, 384], BF16, isOutput=False)
    wv = nc.declare_dram_parameter("wv", [128, 6, 192], BF16, isOutput=False)
    w2 = nc.declare_dram_parameter("w2", [192, D], BF16, isOutput=False)
    out = nc.declare_dram_parameter("out", [S, D], BF16, isOutput=True)

    with tile.TileContext(nc) as tc:
        with (
            tc.tile_pool(name="persist", bufs=1) as pers,
            tc.tile_pool(name="ptpool", bufs=4) as ptpool,
            tc.tile_pool(name="norm", bufs=3) as norm,
            tc.tile_pool(name="stage", bufs=3) as stage,
            tc.tile_pool(name="ps_sc", bufs=2, space="PSUM") as ps_sc,
            tc.tile_pool(name="ps_otp", bufs=3, space="PSUM") as ps_otp,
            tc.tile_pool(name="ps_fill", bufs=1, space="PSUM") as ps_fill,
        ):
            # ---- persistent SBUF tiles ----
            xt0_sb = [pers.tile([128, 512], BF16, tag=f"xt0_{k}", name=f"xt0_{k}")
                      for k in range(6)]
            xtb_sb = [pers.tile([128, 6, 512], BF16, tag=f"xtb{t}", name=f"xtb{t}")
                      for t in (1, 2, 3)]
            wqka_sb = pers.tile([128, 3, 384], BF16, tag="wqka", name="wqka")
            wqkb_sb = pers.tile([128, 3, 384], BF16, tag="wqkb", name="wqkb")
            wv_sb = pers.tile([128, 6, 192], BF16, tag="wv", name="wv")
            w2a_sb = pers.tile([128, D], BF16, tag="w2a")
            w2b_sb = pers.tile([64, D], BF16, tag="w2b")
            # all attention-side operands are split into per-512-block
            # tiles so writes for block nt never create coarse-grained
            # false dependencies against reads of other blocks
            q01s = [pers.tile([128, 512], BF16, tag=f"q01_{i}", name=f"q01_{i}")
                    for i in range(NSUP)]   # [Q0.T ; Q1.T] per q-superblock
            k01t = [pers.tile([128, 512], BF16, tag=f"k01_{i}", name=f"k01_{i}")
                    for i in range(NSUP)]   # [K0.T ; K1.T] per t-block
            q2ds = [pers.tile([128, 512], BF16, tag=f"q2d_{i}", name=f"q2d_{i}")
                    for i in range(NSUP)]   # [Q2.T ; Q2.T] per q-superblock
            k2dt = [pers.tile([128, 512], BF16, tag=f"k2d_{i}", name=f"k2d_{i}")
                    for i in range(NSUP)]   # [K2.T ; K2.T] per t-block
            # V natural + ones column: [t-part, block, head, 65] per t-block
            vnt = [pers.tile([128, 4, H_PER_CORE, 65], BF16, tag=f"vn_{i}",
                             name=f"vn_{i}") for i in range(NSUP)]
            ot01s = [pers.tile([128, 512], BF16, tag=f"ot01_{i}",
                               name=f"ot01_{i}") for i in range(NSUP)]
            ot2s = [pers.tile([64, 512], BF16, tag=f"ot2_{i}", name=f"ot2_{i}")
                    for i in range(NSUP)]

            # warm-up source first in the DVE queue (gates the first matmul)
            wsrc = pers.tile([128, 512], BF16, tag="wsrc")
            nc.vector.memset(wsrc, 1.0)
            # exp table preload: tiny dummy activation during the DMA wait
            dumm = pers.tile([1, 16], F32, tag="dumm")
            dumo = pers.tile([1, 16], BF16, tag="dumo")
            nc.vector.memset(dumm, 0.0)
            nc.scalar.activation(out=dumo, in_=dumm,
                                 func=mybir.ActivationFunctionType.Exp, scale=1.0)
            # dummy broadcast: forces the gpsimd "attn" ucode library to
            # load now (~7us) instead of at the first real broadcast
            dumb = pers.tile([2, 16], F32, tag="dumb")
            nc.gpsimd.partition_broadcast(dumb, dumm)
            for i in range(NSUP):
                nc.vector.memset(vnt[i][:, :, :, 64:65], 1.0)

            # ---- input DMAs: critical-first on both HWDGE queues.
            # xt0 per-chunk so qkv0's k-outer matmuls start on chunk 0.
            nc.scalar.dma_start(out=wqka_sb, in_=wqka[:, :, :])
            for k in range(6):
                nc.sync.dma_start(out=xt0_sb[k], in_=xtb0[k][:, :])
            nc.scalar.dma_start(out=wqkb_sb, in_=wqkb[:, :, :])
            nc.scalar.dma_start(out=wv_sb, in_=wv[:, :, :])
            nc.scalar.dma_start(out=xtb_sb[0], in_=xtb1[:, :, :])
            nc.sync.dma_start(out=xtb_sb[1], in_=xtb2[:, :, :])
            nc.scalar.dma_start(out=xtb_sb[2], in_=xtb3[:, :, :])
            nc.sync.dma_start(out=w2a_sb, in_=w2[0:128, :])
            nc.scalar.dma_start(out=w2b_sb, in_=w2[128:192, :])

            def xt_ap(nt, k):
                # [128, 512] moving slice of chunk k, t-block nt
                if nt == 0:
                    return xt0_sb[k]
                return xtb_sb[nt - 1][:, k, :]

            def wqk_ap(k, c):
                t = wqka_sb if k < 3 else wqkb_sb
                return t[:, k % 3, 128 * c: 128 * c + 128]

            # PE warm-up: keeps the HAM activity monitor busy during the
            # input DMA wait (result discarded)
            wps = ps_otp.tile([128, 512], F32, tag="otp", name="wps")
            for wi in range(7):
                nc.tensor.matmul(
                    wps, wsrc[:, 0:128], wsrc,
                    start=(wi == 0), stop=(wi == 6), skip_group_check=True,
                )

            # ---- QKV evacuation helpers ----
            def evac_qk(c, pt, nt):
                if c == 0:
                    nc.vector.tensor_copy(q01s[nt], pt)
                elif c == 1:
                    nc.vector.tensor_copy(k01t[nt], pt)
                else:
                    nc.vector.tensor_copy(q2ds[nt][0:64, :], pt[0:64, :])
                    nc.vector.tensor_copy(k2dt[nt][0:64, :], pt[64:128, :])
                    # duplicates stay on DVE: gpsimd.tensor_copy lives in
                    # the "standard" ucode library while partition_broadcast
                    # is in "attn" -- mixing them thrashes a ~6us lib reload
                    nc.vector.tensor_copy(q2ds[nt][64:128, :], pt[0:64, :])
                    nc.vector.tensor_copy(k2dt[nt][64:128, :], pt[64:128, :])

            # ---- QKV for t-block 0: k-outer so matmuls start on chunk 0 ----
            def emit_qkv0():
                sc0 = ps_sc.tile([128, 2, 512], F32, tag="sc", name="qsc0")
                sc1 = ps_sc.tile([128, 2, 512], F32, tag="sc", name="qsc1")
                cts = [sc0[:, 0, :], sc0[:, 1, :], sc1[:, 0, :]]
                for k in range(NKCH):
                    st, sp = k == 0, k == NKCH - 1
                    for c in range(3):
                        nc.tensor.matmul(
                            cts[c], wqk_ap(k, c), xt_ap(0, k),
                            start=st, stop=sp, skip_group_check=True,
                        )
                # q/k evacuations first so sup0's scores (and the exp
                # stream) can start immediately; V runs as sup0 fillers
                for c in range(3):
                    evac_qk(c, cts[c], 0)

            def unit_v0(half):
                def emit():
                    po = ps_fill.tile([128, 512], F32, tag="fill", name="uv0")
                    for i in range(2):
                        b_ = 2 * half + i
                        pv = po[:, 192 * i: 192 * i + 192]
                        for k in range(NKCH):
                            nc.tensor.matmul(
                                pv, xt_ap(0, k)[:, bass.ts(b_, 128)],
                                wv_sb[:, k, :], start=(k == 0),
                                stop=(k == NKCH - 1),
                            )
                    nc.vector.tensor_copy(vnt[0][:, 2 * half: 2 * half + 2, :, 0:64],
                                          po[:, 0:384])
                return emit

            # ---- QKV filler units for t-block nt (1..3) ----
            def unit_cgroup(nt, c):
                def emit():
                    pt = ps_fill.tile([128, 512], F32, tag="fill", name="ucg")
                    for k in range(NKCH):
                        nc.tensor.matmul(
                            pt, wqk_ap(k, c), xt_ap(nt, k),
                            start=(k == 0), stop=(k == NKCH - 1),
                        )
                    evac_qk(c, pt, nt)
                return emit

            def unit_vgroup(nt, half, borrow):
                def emit():
                    blk = 4 * nt + 2 * half
                    pool = ps_otp if borrow else ps_fill
                    po = pool.tile([128, 512], F32,
                                   tag="otp" if borrow else "fill", name="uvg")
                    for i in range(2):
                        pv = po[:, 192 * i: 192 * i + 192]
                        for k in range(NKCH):
                            nc.tensor.matmul(
                                pv, xt_ap(nt, k)[:, bass.ts(blk + i - 4 * nt, 128)],
                                wv_sb[:, k, :],
                                start=(k == 0), stop=(k == NKCH - 1),
                            )
                    nc.vector.tensor_copy(
                        vnt[nt][:, 2 * half: 2 * half + 2, :, 0:64],
                        po[:, 0:384])
                return emit

            # ---- output projection halves as filler units ----
            def unit_op(qb, half, borrow):
                def emit():
                    qs = bass.ts(qb % 4, 128)      # slice within the sup tile
                    qs_out = bass.ts(qb, 128)      # global output rows
                    pool = ps_otp if borrow else ps_fill
                    pt = pool.tile([128, 512], F32,
                                   tag="otp" if borrow else "fill", name="uop")
                    ncol, nlen = (0, 512) if half == 0 else (512, 256)
                    dst = pt if half == 0 else pt[:, 0:256]
                    nc.tensor.matmul(dst, ot01s[qb // 4][:, qs],
                                     w2a_sb[:, ncol: ncol + nlen],
                                     start=True, stop=False)
                    nc.tensor.matmul(dst, ot2s[qb // 4][:, qs],
                                     w2b_sb[:, ncol: ncol + nlen],
                                     start=False, stop=True)
                    if half == 0:
                        ostg = stage.tile([128, D], BF16, tag="ostage",
                                          name=f"ostg{qb}")
                        _ostage[qb] = ostg
                        nc.vector.tensor_copy(ostg[:, 0:512], dst)
                    else:
                        ostg = _ostage[qb]
                        nc.vector.tensor_copy(ostg[:, 512:768], dst)
                        # all outputs on the sync queue: a doorbell on the
                        # scalar engine would stall the exp stream
                        nc.sync.dma_start(out=out[qs_out, :], in_=ostg)
                return emit

            _ostage = {}

            # ---- norm helpers ----
            def emit_norm_l(otp):
                # copy l row, reciprocal; broadcast+multiply deferred
                lt = norm.tile([1, 512], F32, tag="lt")
                nc.vector.tensor_copy(lt, otp[64:65, :])
                rlt = norm.tile([1, 512], F32, tag="rlt")
                nc.vector.reciprocal_approx_fast(out=rlt, in_=lt)
                return rlt

            def emit_norm_bcast(rlt):
                rbc = norm.tile([64, 512], F32, tag="rbc")
                nc.gpsimd.partition_broadcast(rbc, rlt)
                return rbc

            def emit_norm_mul(otp, rbc, dst):
                # multiply straight from PSUM; this read releases the bank
                nc.vector.tensor_mul(dst, otp[0:64, :], rbc)

            # ---- the pair stream for one super-block ----
            def emit_sup(sup, otp0, otp1, otp2, fill_at, post_slot_hooks):
                """fill_at: dict slot -> [filler callables] run after that
                slot's PV.  post_slot_hooks: dict slot -> [callables] run
                after the fillers (norm emissions / deferred muls)."""
                nch = 4 * sup + 4
                # pair descriptors: (jA, jB, use_h2, vnheads, stream_id)
                pairs = [(j, j, False, (0, 1), 0) for j in range(nch)]
                pairs += [(j, j + 1, True, (2, 2), 1)
                          for j in range(0, nch, 2)]
                n = len(pairs)
                otps = {0: (otp0, otp1), 1: (otp2, otp2)}
                n01 = nch  # number of h01 pairs
                n2 = n - n01

                ptiles = {}

                def emit_score(i):
                    jA, jB, h2, _, _ = pairs[i]
                    c0A = max(0, 128 * (jA - 4 * sup))
                    c0B = max(0, 128 * (jB - 4 * sup))
                    ktA = (k2dt if h2 else k01t)[jA // 4]
                    ktB = (k2dt if h2 else k01t)[jB // 4]
                    qt = (q2ds if h2 else q01s)[sup]
                    sc = ps_sc.tile([128, 2, 512], F32, tag="sc")
                    nc.tensor.matmul(
                        sc[:, 0, c0A:512], ktA[0:64, bass.ts(jA % 4, 128)],
                        qt[0:64, c0A:512],
                        start=True, stop=True, tile_position=(0, 0),
                    )
                    nc.tensor.matmul(
                        sc[:, 1, c0B:512], ktB[64:128, bass.ts(jB % 4, 128)],
                        qt[64:128, c0B:512],
                        start=True, stop=True, tile_position=(64, 0),
                    )
                    ptile = ptpool.tile([128, 2, 512], BF16, tag="pt")
                    nc.scalar.activation(
                        out=ptile[:, :, c0A:512], in_=sc[:, :, c0A:512],
                        func=mybir.ActivationFunctionType.Exp, scale=SCALE,
                    )
                    if jB >= 4 * sup:  # diagonal: zero triangles (+ B stale)
                        if jA == jB - 1:
                            nc.gpsimd.affine_select(
                                out=ptile[:, :, c0A: c0A + 256],
                                in_=ptile[:, :, c0A: c0A + 256],
                                pattern=[[-128, 2], [1, 256]],
                                compare_op=mybir.AluOpType.is_ge,
                                fill=0.0, base=0, channel_multiplier=-1,
                            )
                        else:
                            nc.gpsimd.affine_select(
                                out=ptile[:, :, c0A: c0A + 128],
                                in_=ptile[:, :, c0A: c0A + 128],
                                pattern=[[0, 2], [1, 128]],
                                compare_op=mybir.AluOpType.is_ge,
                                fill=0.0, base=0, channel_multiplier=-1,
                            )
                    ptiles[i] = ptile

                def emit_pv(i):
                    jA, jB, h2, vn_h, sid = pairs[i]
                    c0A = max(0, 128 * (jA - 4 * sup))
                    c0B = max(0, 128 * (jB - 4 * sup))
                    ptile = ptiles.pop(i)
                    oA, oB = otps[sid]
                    if sid == 0:
                        i0 = i
                        startA = startB = i0 == 0
                        stopA = stopB = i0 == n01 - 1
                    else:
                        i0 = i - n01
                        startA = i0 == 0
                        stopA = False
                        startB = False
                        stopB = i0 == n2 - 1
                    nc.tensor.matmul(
                        oA[0:65, c0A:512], vnt[jA // 4][:, jA % 4, vn_h[0], :],
                        ptile[:, 0, c0A:512],
                        start=startA, stop=stopA, skip_group_check=True,
                    )
                    nc.tensor.matmul(
                        oB[0:65, c0B:512], vnt[jB // 4][:, jB % 4, vn_h[1], :],
                        ptile[:, 1, c0B:512],
                        start=startB, stop=stopB, skip_group_check=True,
                    )

                emit_score(0)
                if n > 1:
                    emit_score(1)
                for f in fill_at.get(-1, []):
                    f()
                # scores clustered two-at-a-time: each cluster pays the
                # row-group LDWEIGHTS serialization once instead of twice
                for i in range(n):
                    if i % 2 == 0:
                        if i + 2 < n:
                            emit_score(i + 2)
                        if i + 3 < n:
                            emit_score(i + 3)
                    emit_pv(i)
                    for f in fill_at.get(i, []):
                        f()
                    for h in post_slot_hooks.get(i, []):
                        h()

            # =================== main schedule ===================
            emit_qkv0()

            deferred_h2 = None  # callable: prev sup's h2 broadcast-multiply
            # global OP-half backlog; per-sup quotas chosen so each sup's
            # PE load (pairs + fillers) roughly matches its ACT load
            op_backlog = [(qb, half) for qb in range(16) for half in (0, 1)]
            op_quota = {0: 0, 1: 5, 2: 7, 3: 12}

            for sup in range(NSUP):
                nch = 4 * sup + 4
                n01 = nch
                n = n01 + nch // 2

                otp0 = ps_otp.tile([128, 512], F32, tag="otp", name=f"otp0_{sup}")
                otp1 = ps_otp.tile([128, 512], F32, tag="otp", name=f"otp1_{sup}")
                otp2 = ps_otp.tile([128, 512], F32, tag="otp", name=f"otp2_{sup}")

                # --- fillers: per-block tiles mean no false deps, so
                # spread everything uniformly; OP units not before slot 3
                # (they need the previous sup's deferred norm multiplies)
                units = []
                if sup < NSUP - 1:
                    nt = sup + 1
                    units += [unit_cgroup(nt, 0), unit_vgroup(nt, 0, False),
                              unit_cgroup(nt, 1), unit_vgroup(nt, 1, False),
                              unit_cgroup(nt, 2)]
                if sup >= 1:
                    op_units = []
                    for qb, half in op_backlog[:op_quota[sup]]:
                        assert qb // 4 < sup
                        op_units.append(unit_op(qb, half, False))
                    op_backlog = op_backlog[op_quota[sup]:]
                    # interleave OP units between qkv units
                    merged = []
                    qi = oi = 0
                    while qi < len(units) or oi < len(op_units):
                        if oi < len(op_units):
                            merged.append(("op", op_units[oi])); oi += 1
                        if oi < len(op_units):
                            merged.append(("op", op_units[oi])); oi += 1
                        if qi < len(units):
                            merged.append(("qkv", units[qi])); qi += 1
                    units = merged
                else:
                    units = [("qkv", u) for u in units]
                fill_at = {}
                nf = len(units)
                # stop fillers ~3 slots before the stream end so the last
                # P.V matmuls (which gate the norm chain and the tail) run
                # immediately after their exps
                hi_slot = max(4, n - 8)
                for idx, (kind, u) in enumerate(units):
                    slot = 1 + (idx * (hi_slot - 1)) // max(nf, 1)
                    if kind == "op":
                        slot = max(slot, 3)
                    fill_at.setdefault(min(slot, hi_slot), []).append(u)
                if sup == 0:
                    # V for t-block 0 must land before the first P.V reads
                    fill_at[-1] = [unit_v0(0), unit_v0(1)]

                hooks = {}
                if deferred_h2 is not None:
                    hooks.setdefault(1, []).append(deferred_h2)
                    deferred_h2 = None

                state = {}

                def mk_h01_norm(o0, o1, st):
                    def h():
                        r0 = emit_norm_l(o0)
                        r1 = emit_norm_l(o1)
                        st["b0"] = emit_norm_bcast(r0)
                        st["b1"] = emit_norm_bcast(r1)
                    return h

                def mk_h01_mul(o0, o1, st, sup_):
                    def h():
                        emit_norm_mul(o0, st["b0"], ot01s[sup_][0:64, :])
                        emit_norm_mul(o1, st["b1"], ot01s[sup_][64:128, :])
                    return h

                hooks.setdefault(n01 - 1, []).append(
                    mk_h01_norm(otp0, otp1, state))
                # the muls ~2 slots later so the broadcast has landed
                hooks.setdefault(min(n - 1, n01 + 1), []).append(
                    mk_h01_mul(otp0, otp1, state, sup))

                emit_sup(sup, otp0, otp1, otp2, fill_at, hooks)

                # h2 norm: l+recip+bcast now; the multiply is deferred into
                # the next sup (slot 0) / the tail
                r2 = emit_norm_l(otp2)
                b2 = emit_norm_bcast(r2)

                def mk_h2_mul(o2, b2_, sup_):
                    def h():
                        emit_norm_mul(o2, b2_, ot2s[sup_])
                    return h

                deferred_h2 = mk_h2_mul(otp2, b2, sup)

            # ---- tail ----
            # The h2 norm chain (l copy -> recip -> broadcast -> multiply)
            # is ~3us serial.  Bridge it with the ot01-half matmuls of the
            # last OP blocks (they only need ot01s[3], already normalized):
            # the score-PSUM banks are free at this point, giving two
            # 1.5-bank homes so the finishing matmuls can overlap.
            def tail_pre(qb):
                qs = bass.ts(qb % 4, 128)
                sct = ps_sc.tile([128, 2, 512], F32, tag="sc", name=f"tl{qb}")
                nc.tensor.matmul(sct[:, 0, :], ot01s[3][:, qs],
                                 w2a_sb[:, 0:512], start=True, stop=False,
                                 skip_group_check=True)
                nc.tensor.matmul(sct[:, 1, 0:256], ot01s[3][:, qs],
                                 w2a_sb[:, 512:768], start=True, stop=False,
                                 skip_group_check=True)
                return sct

            def tail_fin(qb, sct):
                qs = bass.ts(qb % 4, 128)
                qs_out = bass.ts(qb, 128)
                nc.tensor.matmul(sct[:, 0, :], ot2s[3][:, qs],
                                 w2b_sb[:, 0:512], start=False, stop=True,
                                 skip_group_check=True)
                nc.tensor.matmul(sct[:, 1, 0:256], ot2s[3][:, qs],
                                 w2b_sb[:, 512:768], start=False, stop=True,
                                 skip_group_check=True)
                ostg = stage.tile([128, D], BF16, tag="ostage", name=f"tlo{qb}")
                # the exp stream is over: the scalar engine is free to help
                # evacuate, and its DMA queue is free for half the outputs
                nc.vector.tensor_copy(ostg[:, 0:512], sct[:, 0, :])
                nc.scalar.copy(ostg[:, 512:768], sct[:, 1, 0:256])
                if qb == 15:
                    # split the last block across both queues so the final
                    # drain is half as long
                    nc.sync.dma_start(out=out[qs_out, :][0:64, :], in_=ostg[0:64, :])
                    nc.scalar.dma_start(out=out[qs_out, :][64:128, :], in_=ostg[64:128, :])
                else:
                    dq = nc.scalar if qb % 2 else nc.sync
                    dq.dma_start(out=out[qs_out, :], in_=ostg)

            # PE keepalive through the norm chain: junk matmuls into the
            # retired fill bank stop the HAM activity monitor from
            # dropping the PE clock to 1.2 GHz during the ~3.4us idle
            kps = ps_fill.tile([128, 512], F32, tag="fill", name="kps")
            for wi in range(8):
                nc.tensor.matmul(kps, wsrc[:, 0:128], wsrc,
                                 start=(wi == 0), stop=(wi == 7),
                                 skip_group_check=True)
            t12 = tail_pre(12)
            t13 = tail_pre(13)
            deferred_h2()
            tail_fin(12, t12)
            t14 = tail_pre(14)
            tail_fin(13, t13)
            t15 = tail_pre(15)
            tail_fin(14, t14)
            tail_fin(15, t15)

    nc.compile()
    return nc


def _get_nc():
    global _CACHED_NC
    if _CACHED_NC is None:
        _CACHED_NC = build_bass()
    return _CACHED_NC


def make_in_maps(x, Wq, Wk, Wv, Wo):
    x = np.asarray(x, dtype=np.float32)
    Wq = np.asarray(Wq, dtype=np.float32)
    Wk = np.asarray(Wk, dtype=np.float32)
    Wv = np.asarray(Wv, dtype=np.float32)
    Wo = np.asarray(Wo, dtype=np.float32)
    bf = ml_dtypes.bfloat16
    in_maps = []
    for c in range(N_CORES):
        b = c // 4
        hs = [H_PER_CORE * (c % 4) + i for i in range(H_PER_CORE)]
        xT_host = np.ascontiguousarray(x[b].T).astype(bf)  # [768, 2048]
        # per 512-col t-block, partition-major: [nt][128, k, 512]
        xtb = xT_host.reshape(NKCH, 128, NSUP, 512).transpose(2, 1, 0, 3)
        wqk_full = np.concatenate(
            [Wq[hs[0]], Wq[hs[1]], Wk[hs[0]], Wk[hs[1]], Wq[hs[2]], Wk[hs[2]]],
            axis=1,
        ).astype(bf)  # [768, 384]
        wqkb_ = wqk_full.reshape(NKCH, 128, 384).transpose(1, 0, 2)
        wv_full = np.concatenate([Wv[h] for h in hs], axis=1).astype(bf)
        wvb = np.ascontiguousarray(
            wv_full.reshape(NKCH, 128, 192).transpose(1, 0, 2))
        w2 = np.ascontiguousarray(np.concatenate(
            [Wo[:, DK * h: DK * h + DK].T for h in hs], axis=0
        ).astype(bf))  # [192, 768]
        in_maps.append({
            **{f"xtb0_{k}": np.ascontiguousarray(xtb[0][:, k])
               for k in range(6)},
            "xtb1": np.ascontiguousarray(xtb[1]),
            "xtb2": np.ascontiguousarray(xtb[2]),
            "xtb3": np.ascontiguousarray(xtb[3]),
            "wqka": np.ascontiguousarray(wqkb_[:, 0:3]),
            "wqkb": np.ascontiguousarray(wqkb_[:, 3:6]),
            "wv": wvb,
            "w2": w2,
        })
    return in_maps


def run_cores(in_maps, **kwargs):
    nc = _get_nc()
    return run_bass_kernel_spmd(nc, in_maps, core_ids=list(range(N_CORES)), **kwargs)


def kernel(x, Wq, Wk, Wv, Wo, bo):
    in_maps = make_in_maps(x, Wq, Wk, Wv, Wo)
    res = run_cores(in_maps)
    bo = np.asarray(bo, dtype=np.float32)
    out = np.empty((B, S, D), dtype=np.float32)
    for b in range(B):
        acc = res.results[4 * b]["out"].astype(np.float32)
        for c in range(4 * b + 1, 4 * b + 4):
            acc = acc + res.results[c]["out"].astype(np.float32)
        out[b] = acc + bo[None, :]
    return out


# revision 36
# speedup vs baseline: 1.0415x; 1.0415x over previous
"""Multi-head causal attention (d_model=768, 12 heads, seq 2048, batch 2) on
8 Trainium2 NeuronCores.

Sharding: tensor-parallel over heads x data-parallel over batch.
Core c handles batch b = c // 4 and heads [3*(c%4), 3*(c%4)+3).
Each core computes its 3 heads' attention plus its partial output
projection; the host sums the 4 partials per batch and adds the bias.

Design (all-bf16 PE datapath), ~115us/core measured:
  - Scores matmuls have only K=64 contraction, so two are packed into
    the 128-row array concurrently via tile_position row groups. Heads
    0,1 pair naturally; head 2 pairs its own chunks j/j+1 against
    partition-duplicated Q2/K2 copies.  Score pairs are emitted
    clustered two-at-a-time, two pairs ahead of the exp stream: each
    cluster pays the row-group LDWEIGHTS serialization (~100ns, no
    background-buffer pull-ahead for tile_position loads) once.
  - Software-pipelined emission: P.V runs one pair behind exp; QKV for
    the next 512-block and output-projection halves are interleaved
    between pairs as PE fillers with per-sup quotas chosen so each
    superblock's PE load matches its ACT load.  The PE never idles
    long enough for the HAM activity monitor to re-throttle the clock
    to 1.2 GHz (it stays at 2.4 GHz for the whole kernel body).
  - All attention-side operands (q01/k01/q2d/k2d/V/ot01/ot2) are split
    into per-512-block tiles so filler writes never create coarse
    whole-tile false dependencies against pair-stream reads.
  - exp runs as one ACTIVATE per pair ([128,2,512] PSUM); diagonal
    triangles are zeroed after exp by gpsimd affine_select.  All
    partition_broadcasts are emitted after the affine_selects they
    would otherwise clog, and a dummy broadcast at build start forces
    the gpsimd "attn" ucode library load (~7us) into the DMA wait.
    gpsimd.tensor_copy is avoided entirely (it lives in the "standard"
    library; mixing would thrash a ~6us library reload per switch).
  - Row sums l come from a ones-column appended to V ([V|1]).
    Normalization per head: copy the l row, reciprocal_approx_fast,
    partition_broadcast (deferred), then one tensor_mul reading O
    directly from PSUM (no staging copy; the read frees the bank).
  - PSUM budget: score pairs 2x[128,2,512] (4 banks) + 3 P.V
    accumulators + 1 filler bank = 8.  V accumulation is block-outer:
    interleaved accumulation groups sharing a PSUM bank corrupt each
    other.
  - Inputs are host-prepped per-512-block chunk-major and DMAed
    critical-first on both HWDGE queues (xT t-block-0 per-chunk so the
    first QKV matmul starts ~3us after the rings open); warm-up
    matmuls hold the PE busy through the DMA wait so qkv0 runs at
    full clock.
  - Tail: the last superblock's h2 norm chain is bridged with
    keepalive matmuls plus ot01-half preloads of the final OP blocks
    into the freed score banks; the scalar engine (exp done) takes
    half the final evacuations and output DMAs.
"""

import sys
import types

import numpy as np
import ml_dtypes

import concourse.bass as bass
import concourse.tile as tile
from concourse import mybir, bacc
from concourse.bass_utils import run_bass_kernel_spmd

# Register the axon NTFF profiling hook if the environment supports it, so
# running with BASS_TRACE=1 yields exec_time_ns instead of an import error.
try:
    import antenv.axon_hooks  # noqa: F401
except ImportError:
    try:
        from trn_agent_boot.trn_boot import _ntff_profile_via_ctypes

        _hook = _ntff_profile_via_ctypes("/opt/axon/libaxon_pjrt.so")
        _mod = types.ModuleType("antenv.axon_hooks")
        _mod.get_axon_ntff_profile_hook = lambda: _hook
        _mod.set_axon_ntff_profile_hook = lambda h: None
        sys.modules["antenv.axon_hooks"] = _mod
    except Exception:
        pass

F32 = mybir.dt.float32
BF16 = mybir.dt.bfloat16

N_CORES = 8
B = 2
S = 2048
D = 768
H = 12
DK = 64
H_PER_CORE = 3
NSUP = S // 512  # 4 q super-blocks of 512
NKCH = D // 128  # 6 contraction chunks
SCALE = 0.125  # 1/sqrt(64)

_CACHED_NC = None


def build_bass():
    nc = bacc.Bacc()
    # inputs, chunk-major per 512-col t-block: xtb*[k][128, 512]
    xtb0 = [nc.declare_dram_parameter(f"xtb0_{k}", [128, 512], BF16,
                                      isOutput=False) for k in range(6)]
    xtb1 = nc.declare_dram_parameter("xtb1", [128, 6, 512], BF16, isOutput=False)
    xtb2 = nc.declare_dram_parameter("xtb2", [128, 6, 512], BF16, isOutput=False)
    xtb3 = nc.declare_dram_parameter("xtb3", [128, 6, 512], BF16, isOutput=False)
    # wqk chunk columns: [Wq0|Wq1], [Wk0|Wk1], [Wq2|Wk2]
    wqka = nc.declare_dram_parameter("wqka", [128, 3, 384], BF16, isOutput=False)
    wqkb = nc.declare_dram_parameter("wqkb", [128, 3, 384], BF16, isOutput=False)
    wv = nc.declare_dram_parameter("wv", [128, 6, 192], BF16, isOutput=False)
    w2 = nc.declare_dram_parameter("w2", [192, D], BF16, isOutput=False)
    out = nc.declare_dram_parameter("out", [S, D], BF16, isOutput=True)

    with tile.TileContext(nc) as tc:
        with (
            tc.tile_pool(name="persist", bufs=1) as pers,
            tc.tile_pool(name="ptpool", bufs=4) as ptpool,
            tc.tile_pool(name="norm", bufs=3) as norm,
            tc.tile_pool(name="stage", bufs=3) as stage,
            tc.tile_pool(name="ps_sc", bufs=2, space="PSUM") as ps_sc,
            tc.tile_pool(name="ps_otp", bufs=3, space="PSUM") as ps_otp,
            tc.tile_pool(name="ps_fill", bufs=1, space="PSUM") as ps_fill,
        ):
            # ---- persistent SBUF tiles ----
            xt0_sb = [pers.tile([128, 512], BF16, tag=f"xt0_{k}", name=f"xt0_{k}")
                      for k in range(6)]
            xtb_sb = [pers.tile([128, 6, 512], BF16, tag=f"xtb{t}", name=f"xtb{t}")
                      for t in (1, 2, 3)]
            wqka_sb = pers.tile([128, 3, 384], BF16, tag="wqka", name="wqka")
            wqkb_sb = pers.tile([128, 3, 384], BF16, tag="wqkb", name="wqkb")
            wv_sb = pers.tile([128, 6, 192], BF16, tag="wv", name="wv")
            w2a_sb = pers.tile([128, D], BF16, tag="w2a")
            w2b_sb = pers.tile([64, D], BF16, tag="w2b")
            # all attention-side operands are split into per-512-block
            # tiles so writes for block nt never create coarse-grained
            # false dependencies against reads of other blocks
            q01s = [pers.tile([128, 512], BF16, tag=f"q01_{i}", name=f"q01_{i}")
                    for i in range(NSUP)]   # [Q0.T ; Q1.T] per q-superblock
            k01t = [pers.tile([128, 512], BF16, tag=f"k01_{i}", name=f"k01_{i}")
                    for i in range(NSUP)]   # [K0.T ; K1.T] per t-block
            q2ds = [pers.tile([128, 512], BF16, tag=f"q2d_{i}", name=f"q2d_{i}")
                    for i in range(NSUP)]   # [Q2.T ; Q2.T] per q-superblock
            k2dt = [pers.tile([128, 512], BF16, tag=f"k2d_{i}", name=f"k2d_{i}")
                    for i in range(NSUP)]   # [K2.T ; K2.T] per t-block
            # V natural + ones column: [t-part, block, head, 65] per t-block
            vnt = [pers.tile([128, 4, H_PER_CORE, 65], BF16, tag=f"vn_{i}",
                             name=f"vn_{i}") for i in range(NSUP)]
            ot01s = [pers.tile([128, 512], BF16, tag=f"ot01_{i}",
                               name=f"ot01_{i}") for i in range(NSUP)]
            ot2s = [pers.tile([64, 512], BF16, tag=f"ot2_{i}", name=f"ot2_{i}")
                    for i in range(NSUP)]

            # warm-up source first in the DVE queue (gates the first matmul)
            wsrc = pers.tile([128, 512], BF16, tag="wsrc")
            nc.vector.memset(wsrc, 1.0)
            # exp table preload: tiny dummy activation during the DMA wait
            dumm = pers.tile([1, 16], F32, tag="dumm")
            dumo = pers.tile([1, 16], BF16, tag="dumo")
            nc.vector.memset(dumm, 0.0)
            nc.scalar.activation(out=dumo, in_=dumm,
                                 func=mybir.ActivationFunctionType.Exp, scale=1.0)
            # dummy broadcast: forces the gpsimd "attn" ucode library to
            # load now (~7us) instead of at the first real broadcast
            dumb = pers.tile([2, 16], F32, tag="dumb")
            nc.gpsimd.partition_broadcast(dumb, dumm)
            for i in range(NSUP):
                nc.vector.memset(vnt[i][:, :, :, 64:65], 1.0)

            # ---- input DMAs: critical-first on both HWDGE queues.
            # xt0 per-chunk so qkv0's k-outer matmuls start on chunk 0.
            nc.scalar.dma_start(out=wqka_sb, in_=wqka[:, :, :])
            for k in range(6):
                nc.sync.dma_start(out=xt0_sb[k], in_=xtb0[k][:, :])
            nc.scalar.dma_start(out=wqkb_sb, in_=wqkb[:, :, :])
            nc.scalar.dma_start(out=wv_sb, in_=wv[:, :, :])
            nc.scalar.dma_start(out=xtb_sb[0], in_=xtb1[:, :, :])
            nc.sync.dma_start(out=xtb_sb[1], in_=xtb2[:, :, :])
            nc.scalar.dma_start(out=xtb_sb[2], in_=xtb3[:, :, :])
            nc.sync.dma_start(out=w2a_sb, in_=w2[0:128, :])
            nc.scalar.dma_start(out=w2b_sb, in_=w2[128:192, :])

            def xt_ap(nt, k):
                # [128, 512] moving slice of chunk k, t-block nt
                if nt == 0:
                    return xt0_sb[k]
                return xtb_sb[nt - 1][:, k, :]

            def wqk_ap(k, c):
                t = wqka_sb if k < 3 else wqkb_sb
                return t[:, k % 3, 128 * c: 128 * c + 128]

            # PE warm-up: keeps the HAM activity monitor busy during the
            # input DMA wait (result discarded)
            wps = ps_otp.tile([128, 512], F32, tag="otp", name="wps")
            for wi in range(7):
                nc.tensor.matmul(
                    wps, wsrc[:, 0:128], wsrc,
                    start=(wi == 0), stop=(wi == 6), skip_group_check=True,
                )

            # ---- QKV evacuation helpers ----
            def evac_qk(c, pt, nt):
                if c == 0:
                    nc.vector.tensor_copy(q01s[nt], pt)
                elif c == 1:
                    nc.vector.tensor_copy(k01t[nt], pt)
                else:
                    nc.vector.tensor_copy(q2ds[nt][0:64, :], pt[0:64, :])
                    nc.vector.tensor_copy(k2dt[nt][0:64, :], pt[64:128, :])
                    # duplicates stay on DVE: gpsimd.tensor_copy lives in
                    # the "standard" ucode library while partition_broadcast
                    # is in "attn" -- mixing them thrashes a ~6us lib reload
                    nc.vector.tensor_copy(q2ds[nt][64:128, :], pt[0:64, :])
                    nc.vector.tensor_copy(k2dt[nt][64:128, :], pt[64:128, :])

            # ---- QKV for t-block 0: k-outer so matmuls start on chunk 0 ----
            def emit_qkv0():
                sc0 = ps_sc.tile([128, 2, 512], F32, tag="sc", name="qsc0")
                sc1 = ps_sc.tile([128, 2, 512], F32, tag="sc", name="qsc1")
                cts = [sc0[:, 0, :], sc0[:, 1, :], sc1[:, 0, :]]
                for k in range(NKCH):
                    st, sp = k == 0, k == NKCH - 1
                    for c in range(3):
                        nc.tensor.matmul(
                            cts[c], wqk_ap(k, c), xt_ap(0, k),
                            start=st, stop=sp, skip_group_check=True,
                        )
                # q/k evacuations first so sup0's scores (and the exp
                # stream) can start immediately; V runs as sup0 fillers
                for c in range(3):
                    evac_qk(c, cts[c], 0)

            def unit_v0(half):
                def emit():
                    po = ps_fill.tile([128, 512], F32, tag="fill", name="uv0")
                    for i in range(2):
                        b_ = 2 * half + i
                        pv = po[:, 192 * i: 192 * i + 192]
                        for k in range(NKCH):
                            nc.tensor.matmul(
                                pv, xt_ap(0, k)[:, bass.ts(b_, 128)],
                                wv_sb[:, k, :], start=(k == 0),
                                stop=(k == NKCH - 1),
                            )
                    nc.vector.tensor_copy(vnt[0][:, 2 * half: 2 * half + 2, :, 0:64],
                                          po[:, 0:384])
                return emit

            # ---- QKV filler units for t-block nt (1..3) ----
            def unit_cgroup(nt, c):
                def emit():
                    pt = ps_fill.tile([128, 512], F32, tag="fill", name="ucg")
                    for k in range(NKCH):
                        nc.tensor.matmul(
                            pt, wqk_ap(k, c), xt_ap(nt, k),
                            start=(k == 0), stop=(k == NKCH - 1),
                        )
                    evac_qk(c, pt, nt)
                return emit

            def unit_vgroup(nt, half, borrow):
                def emit():
                    blk = 4 * nt + 2 * half
                    pool = ps_otp if borrow else ps_fill
                    po = pool.tile([128, 512], F32,
                                   tag="otp" if borrow else "fill", name="uvg")
                    for i in range(2):
                        pv = po[:, 192 * i: 192 * i + 192]
                        for k in range(NKCH):
                            nc.tensor.matmul(
                                pv, xt_ap(nt, k)[:, bass.ts(blk + i - 4 * nt, 128)],
                                wv_sb[:, k, :],
                                start=(k == 0), stop=(k == NKCH - 1),
                            )
                    nc.vector.tensor_copy(
                        vnt[nt][:, 2 * half: 2 * half + 2, :, 0:64],
                        po[:, 0:384])
                return emit

            # ---- output projection halves as filler units ----
            def unit_op(qb, half, borrow):
                def emit():
                    qs = bass.ts(qb % 4, 128)      # slice within the sup tile
                    qs_out = bass.ts(qb, 128)      # global output rows
                    pool = ps_otp if borrow else ps_fill
                    pt = pool.tile([128, 512], F32,
                                   tag="otp" if borrow else "fill", name="uop")
                    ncol, nlen = (0, 512) if half == 0 else (512, 256)
                    dst = pt if half == 0 else pt[:, 0:256]
                    nc.tensor.matmul(dst, ot01s[qb // 4][:, qs],
                                     w2a_sb[:, ncol: ncol + nlen],
                                     start=True, stop=False)
                    nc.tensor.matmul(dst, ot2s[qb // 4][:, qs],
                                     w2b_sb[:, ncol: ncol + nlen],
                                     start=False, stop=True)
                    if half == 0:
                        ostg = stage.tile([128, D], BF16, tag="ostage",
                                          name=f"ostg{qb}")
                        _ostage[qb] = ostg
                        nc.vector.tensor_copy(ostg[:, 0:512], dst)
                    else:
                        ostg = _ostage[qb]
                        nc.vector.tensor_copy(ostg[:, 512:768], dst)
                        # all outputs on the sync queue: a doorbell on the
                        # scalar engine would stall the exp stream
                        nc.sync.dma_start(out=out[qs_out, :], in_=ostg)
                return emit

            _ostage = {}

            # ---- norm helpers ----
            def emit_norm_l(otp):
                # copy l row, reciprocal; broadcast+multiply deferred
                lt = norm.tile([1, 512], F32, tag="lt")
                nc.vector.tensor_copy(lt, otp[64:65, :])
                rlt = norm.tile([1, 512], F32, tag="rlt")
                nc.vector.reciprocal_approx_fast(out=rlt, in_=lt)
                return rlt

            def emit_norm_bcast(rlt):
                rbc = norm.tile([64, 512], F32, tag="rbc")
                nc.gpsimd.partition_broadcast(rbc, rlt)
                return rbc

            def emit_norm_mul(otp, rbc, dst):
                # multiply straight from PSUM; this read releases the bank
                nc.vector.tensor_mul(dst, otp[0:64, :], rbc)

            # ---- the pair stream for one super-block ----
            def emit_sup(sup, otp0, otp1, otp2, fill_at, post_slot_hooks):
                """fill_at: dict slot -> [filler callables] run after that
                slot's PV.  post_slot_hooks: dict slot -> [callables] run
                after the fillers (norm emissions / deferred muls)."""
                nch = 4 * sup + 4
                # pair descriptors: (jA, jB, use_h2, vnheads, stream_id)
                pairs = [(j, j, False, (0, 1), 0) for j in range(nch)]
                pairs += [(j, j + 1, True, (2, 2), 1)
                          for j in range(0, nch, 2)]
                n = len(pairs)
                otps = {0: (otp0, otp1), 1: (otp2, otp2)}
                n01 = nch  # number of h01 pairs
                n2 = n - n01

                ptiles = {}

                def emit_score(i):
                    jA, jB, h2, _, _ = pairs[i]
                    c0A = max(0, 128 * (jA - 4 * sup))
                    c0B = max(0, 128 * (jB - 4 * sup))
                    ktA = (k2dt if h2 else k01t)[jA // 4]
                    ktB = (k2dt if h2 else k01t)[jB // 4]
                    qt = (q2ds if h2 else q01s)[sup]
                    sc = ps_sc.tile([128, 2, 512], F32, tag="sc")
                    nc.tensor.matmul(
                        sc[:, 0, c0A:512], ktA[0:64, bass.ts(jA % 4, 128)],
                        qt[0:64, c0A:512],
                        start=True, stop=True, tile_position=(0, 0),
                    )
                    nc.tensor.matmul(
                        sc[:, 1, c0B:512], ktB[64:128, bass.ts(jB % 4, 128)],
                        qt[64:128, c0B:512],
                        start=True, stop=True, tile_position=(64, 0),
                    )
                    ptile = ptpool.tile([128, 2, 512], BF16, tag="pt")
                    nc.scalar.activation(
                        out=ptile[:, :, c0A:512], in_=sc[:, :, c0A:512],
                        func=mybir.ActivationFunctionType.Exp, scale=SCALE,
                    )
                    if jB >= 4 * sup:  # diagonal: zero triangles (+ B stale)
                        if jA == jB - 1:
                            nc.gpsimd.affine_select(
                                out=ptile[:, :, c0A: c0A + 256],
                                in_=ptile[:, :, c0A: c0A + 256],
                                pattern=[[-128, 2], [1, 256]],
                                compare_op=mybir.AluOpType.is_ge,
                                fill=0.0, base=0, channel_multiplier=-1,
                            )
                        else:
                            nc.gpsimd.affine_select(
                                out=ptile[:, :, c0A: c0A + 128],
                                in_=ptile[:, :, c0A: c0A + 128],
                                pattern=[[0, 2], [1, 128]],
                                compare_op=mybir.AluOpType.is_ge,
                                fill=0.0, base=0, channel_multiplier=-1,
                            )
                    ptiles[i] = ptile

                def emit_pv(i):
                    jA, jB, h2, vn_h, sid = pairs[i]
                    c0A = max(0, 128 * (jA - 4 * sup))
                    c0B = max(0, 128 * (jB - 4 * sup))
                    ptile = ptiles.pop(i)
                    oA, oB = otps[sid]
                    if sid == 0:
                        i0 = i
                        startA = startB = i0 == 0
                        stopA = stopB = i0 == n01 - 1
                    else:
                        i0 = i - n01
                        startA = i0 == 0
                        stopA = False
                        startB = False
                        stopB = i0 == n2 - 1
                    nc.tensor.matmul(
                        oA[0:65, c0A:512], vnt[jA // 4][:, jA % 4, vn_h[0], :],
                        ptile[:, 0, c0A:512],
                        start=startA, stop=stopA, skip_group_check=True,
                    )
                    nc.tensor.matmul(
                        oB[0:65, c0B:512], vnt[jB // 4][:, jB % 4, vn_h[1], :],
                        ptile[:, 1, c0B:512],
                        start=startB, stop=stopB, skip_group_check=True,
                    )

                emit_score(0)
                if n > 1:
                    emit_score(1)
                for f in fill_at.get(-1, []):
                    f()
                # scores clustered two-at-a-time: each cluster pays the
                # row-group LDWEIGHTS serialization once instead of twice
                for i in range(n):
                    if i % 2 == 0:
                        if i + 2 < n:
                            emit_score(i + 2)
                        if i + 3 < n:
                            emit_score(i + 3)
                    emit_pv(i)
                    for f in fill_at.get(i, []):
                        f()
                    for h in post_slot_hooks.get(i, []):
                        h()

            # =================== main schedule ===================
            emit_qkv0()

            deferred_h2 = None  # callable: prev sup's h2 broadcast-multiply
            # global OP-half backlog; per-sup quotas chosen so each sup's
            # PE load (pairs + fillers) roughly matches its ACT load
            op_backlog = [(qb, half) for qb in range(16) for half in (0, 1)]
            op_quota = {0: 0, 1: 5, 2: 7, 3: 12}

            for sup in range(NSUP):
                nch = 4 * sup + 4
                n01 = nch
                n = n01 + nch // 2

                otp0 = ps_otp.tile([128, 512], F32, tag="otp", name=f"otp0_{sup}")
                otp1 = ps_otp.tile([128, 512], F32, tag="otp", name=f"otp1_{sup}")
                otp2 = ps_otp.tile([128, 512], F32, tag="otp", name=f"otp2_{sup}")

                # --- fillers: per-block tiles mean no false deps, so
                # spread everything uniformly; OP units not before slot 3
                # (they need the previous sup's deferred norm multiplies)
                units = []
                if sup < NSUP - 1:
                    nt = sup + 1
                    units += [unit_cgroup(nt, 0), unit_vgroup(nt, 0, False),
                              unit_cgroup(nt, 1), unit_vgroup(nt, 1, False),
                              unit_cgroup(nt, 2)]
                if sup >= 1:
                    op_units = []
                    for qb, half in op_backlog[:op_quota[sup]]:
                        assert qb // 4 < sup
                        op_units.append(unit_op(qb, half, False))
                    op_backlog = op_backlog[op_quota[sup]:]
                    # interleave OP units between qkv units
                    merged = []
                    qi = oi = 0
                    while qi < len(units) or oi < len(op_units):
                        if oi < len(op_units):
                            merged.append(("op", op_units[oi])); oi += 1
                        if oi < len(op_units):
                            merged.append(("op", op_units[oi])); oi += 1
                        if qi < len(units):
                            merged.append(("qkv", units[qi])); qi += 1
                    units = merged
                else:
                    units = [("qkv", u) for u in units]
                fill_at = {}
                nf = len(units)
                # stop fillers ~3 slots before the stream end so the last
                # P.V matmuls (which gate the norm chain and the tail) run
                # immediately after their exps
                hi_slot = max(4, n - 6)
                for idx, (kind, u) in enumerate(units):
                    slot = 1 + (idx * (hi_slot - 1)) // max(nf, 1)
                    if kind == "op":
                        slot = max(slot, 3)
                    fill_at.setdefault(min(slot, hi_slot), []).append(u)
                if sup == 0:
                    # V for t-block 0 must land before the first P.V reads
                    fill_at[-1] = [unit_v0(0), unit_v0(1)]

                hooks = {}
                if deferred_h2 is not None:
                    hooks.setdefault(1, []).append(deferred_h2)
                    deferred_h2 = None

                state = {}

                def mk_h01_norm(o0, o1, st):
                    def h():
                        r0 = emit_norm_l(o0)
                        r1 = emit_norm_l(o1)
                        st["b0"] = emit_norm_bcast(r0)
                        st["b1"] = emit_norm_bcast(r1)
                    return h

                def mk_h01_mul(o0, o1, st, sup_):
                    def h():
                        emit_norm_mul(o0, st["b0"], ot01s[sup_][0:64, :])
                        emit_norm_mul(o1, st["b1"], ot01s[sup_][64:128, :])
                    return h

                hooks.setdefault(n01 - 1, []).append(
                    mk_h01_norm(otp0, otp1, state))
                # the muls ~2 slots later so the broadcast has landed
                hooks.setdefault(min(n - 1, n01 + 1), []).append(
                    mk_h01_mul(otp0, otp1, state, sup))

                emit_sup(sup, otp0, otp1, otp2, fill_at, hooks)

                # h2 norm: l+recip+bcast now; the multiply is deferred into
                # the next sup (slot 0) / the tail
                r2 = emit_norm_l(otp2)
                b2 = emit_norm_bcast(r2)

                def mk_h2_mul(o2, b2_, sup_):
                    def h():
                        emit_norm_mul(o2, b2_, ot2s[sup_])
                    return h

                deferred_h2 = mk_h2_mul(otp2, b2, sup)

            # ---- tail ----
            # The h2 norm chain (l copy -> recip -> broadcast -> multiply)
            # is ~3us serial.  Bridge it with the ot01-half matmuls of the
            # last OP blocks (they only need ot01s[3], already normalized):
            # the score-PSUM banks are free at this point, giving two
            # 1.5-bank homes so the finishing matmuls can overlap.
            def tail_pre(qb):
                qs = bass.ts(qb % 4, 128)
                sct = ps_sc.tile([128, 2, 512], F32, tag="sc", name=f"tl{qb}")
                nc.tensor.matmul(sct[:, 0, :], ot01s[3][:, qs],
                                 w2a_sb[:, 0:512], start=True, stop=False,
                                 skip_group_check=True)
                nc.tensor.matmul(sct[:, 1, 0:256], ot01s[3][:, qs],
                                 w2a_sb[:, 512:768], start=True, stop=False,
                                 skip_group_check=True)
                return sct

            def tail_fin(qb, sct):
                qs = bass.ts(qb % 4, 128)
                qs_out = bass.ts(qb, 128)
                nc.tensor.matmul(sct[:, 0, :], ot2s[3][:, qs],
                                 w2b_sb[:, 0:512], start=False, stop=True,
                                 skip_group_check=True)
                nc.tensor.matmul(sct[:, 1, 0:256], ot2s[3][:, qs],
                                 w2b_sb[:, 512:768], start=False, stop=True,
                                 skip_group_check=True)
                ostg = stage.tile([128, D], BF16, tag="ostage", name=f"tlo{qb}")
                # the exp stream is over: the scalar engine is free to help
                # evacuate, and its DMA queue is free for half the outputs
                nc.vector.tensor_copy(ostg[:, 0:512], sct[:, 0, :])
                nc.scalar.copy(ostg[:, 512:768], sct[:, 1, 0:256])
                if qb == 15:
                    # split the last block across both queues so the final
                    # drain is half as long
                    nc.sync.dma_start(out=out[qs_out, :][0:64, :], in_=ostg[0:64, :])
                    nc.scalar.dma_start(out=out[qs_out, :][64:128, :], in_=ostg[64:128, :])
                else:
                    dq = nc.scalar if qb % 2 else nc.sync
                    dq.dma_start(out=out[qs_out, :], in_=ostg)

            # PE keepalive through the norm chain: junk matmuls into the
            # retired fill bank stop the HAM activity monitor from
            # dropping the PE clock to 1.2 GHz during the ~3.4us idle
            kps = ps_fill.tile([128, 512], F32, tag="fill", name="kps")
            for wi in range(8):
                nc.tensor.matmul(kps, wsrc[:, 0:128], wsrc,
                                 start=(wi == 0), stop=(wi == 7),
                                 skip_group_check=True)
            t12 = tail_pre(12)
            t13 = tail_pre(13)
            deferred_h2()
            tail_fin(12, t12)
            t14 = tail_pre(14)
            tail_fin(13, t13)
            t15 = tail_pre(15)
            tail_fin(14, t14)
            tail_fin(15, t15)

    nc.compile()
    return nc


def _get_nc():
    global _CACHED_NC
    if _CACHED_NC is None:
        _CACHED_NC = build_bass()
    return _CACHED_NC


def make_in_maps(x, Wq, Wk, Wv, Wo):
    x = np.asarray(x, dtype=np.float32)
    Wq = np.asarray(Wq, dtype=np.float32)
    Wk = np.asarray(Wk, dtype=np.float32)
    Wv = np.asarray(Wv, dtype=np.float32)
    Wo = np.asarray(Wo, dtype=np.float32)
    bf = ml_dtypes.bfloat16
    in_maps = []
    for c in range(N_CORES):
        b = c // 4
        hs = [H_PER_CORE * (c % 4) + i for i in range(H_PER_CORE)]
        xT_host = np.ascontiguousarray(x[b].T).astype(bf)  # [768, 2048]
        # per 512-col t-block, partition-major: [nt][128, k, 512]
        xtb = xT_host.reshape(NKCH, 128, NSUP, 512).transpose(2, 1, 0, 3)
        wqk_full = np.concatenate(
            [Wq[hs[0]], Wq[hs[1]], Wk[hs[0]], Wk[hs[1]], Wq[hs[2]], Wk[hs[2]]],
            axis=1,
        ).astype(bf)  # [768, 384]
        wqkb_ = wqk_full.reshape(NKCH, 128, 384).transpose(1, 0, 2)
        wv_full = np.concatenate([Wv[h] for h in hs], axis=1).astype(bf)
        wvb = np.ascontiguousarray(
            wv_full.reshape(NKCH, 128, 192).transpose(1, 0, 2))
        w2 = np.ascontiguousarray(np.concatenate(
            [Wo[:, DK * h: DK * h + DK].T for h in hs], axis=0
        ).astype(bf))  # [192, 768]
        in_maps.append({
            **{f"xtb0_{k}": np.ascontiguousarray(xtb[0][:, k])
               for k in range(6)},
            "xtb1": np.ascontiguousarray(xtb[1]),
            "xtb2": np.ascontiguousarray(xtb[2]),
            "xtb3": np.ascontiguousarray(xtb[3]),
            "wqka": np.ascontiguousarray(wqkb_[:, 0:3]),
            "wqkb": np.ascontiguousarray(wqkb_[:, 3:6]),
            "wv": wvb,
            "w2": w2,
        })
    return in_maps


def run_cores(in_maps, **kwargs):
    nc = _get_nc()
    return run_bass_kernel_spmd(nc, in_maps, core_ids=list(range(N_CORES)), **kwargs)


def kernel(x, Wq, Wk, Wv, Wo, bo):
    in_maps = make_in_maps(x, Wq, Wk, Wv, Wo)
    res = run_cores(in_maps)
    bo = np.asarray(bo, dtype=np.float32)
    out = np.empty((B, S, D), dtype=np.float32)
    for b in range(B):
        acc = res.results[4 * b]["out"].astype(np.float32)
        for c in range(4 * b + 1, 4 * b + 4):
            acc = acc + res.results[c]["out"].astype(np.float32)
        out[b] = acc + bo[None, :]
    return out
